# revision 36
# baseline (speedup 1.0000x reference)
"""Trainium2 Bass kernel for nn_HardestContrastiveLoss.

Strategy (1D row-parallel cdist, per sharding hint):
  - Host: gather the selected correspondences (indexing/transpose + exact
    constant scaling + dtype cast), shard 8192 selected rows as 1024/core.
  - Device (per core, identical program, different data):
      * prep: rigid-transform gathered src points (small matmul + fused
        add/scale), square passes + ones-matmuls for the norm terms
      * two matmuls per [128, 512] tile (PE pinned at 1.2 GHz on this
        system -- HAM never un-throttles -- so matmul cost is
        streaming-bound at ~1 col/cycle):
          psf  = -2a.b + |a|^2 + |b|^2  (feats, bf16, K=34)
          psp1 = V*(C1 - |p-q|^2)       (threshold-folded pts, f32r, K=6)
        Feats ship as bf16 (halves the dominant DMA volume -- the 8 cores
        share the chip DMA engines, so input landing time is
        bytes-bound -- and enables fast-weight-load).  With V=1e13 the
        fp32 pts accumulation quantizes psp1 so every pos/neg gap is far
        larger than any feats distance^2: an elementwise min/max against
        psf is an exact mask-select.
      * per [128, 1024] macro tile (column-major (n, m) order so the tgt
        DMA/prep pipeline hides behind compute):
          Act stages psf -> fsb; DVE runs ONE fused custom op per side
          (select + free-dim reduce + seeded accum in a single pass):
            pos: accum[c] = max(0,   max_k min(psp1, fd2))
            neg: accum[c] = min(BIG, min_k max(psp1 + D, fd2))
      * tail: per-m reduces, clamp, sqrt, relu thresholds, ones-matmul
  - Host: sum the 8 per-core [2,1] partials, divide by N (the "all-reduce").

Operand layout (base partition 0 for both matmuls):
  srcF/tgtF [34, .] bf16:  0:32 = -2a (host-scaled) / b;  32 = |a|^2
    (dev) / 1;  33 = 1 / |b|^2 (dev)
  srcP/tgtP [8, .] f32r:   0:3 = 2V*(R p + t) (dev) / q;  3 = -V|p^|^2
    (dev) / 1;  4 = -V / |q|^2 (dev);  5 = 1 / V*C1
"""

import numpy as np

N_SEL = 8192
N_CORES = 8
ROWS_PER_CORE = N_SEL // N_CORES  # 1024
M_TILES = ROWS_PER_CORE // 128  # 8
NT = 512  # matmul tile (one PSUM bank)
GNT = 1024  # macro tile (two PSUM banks)
GN_TILES = N_SEL // GNT  # 8
N_CHUNKS = N_SEL // NT  # 16
KF = 34  # feats rows (-2a | norms)
KP = 8   # pts rows

EPS = 1e-7
POS_RADIUS = 0.0375
NEG_RADIUS = 0.1
POS_THRESH = 0.1
NEG_THRESH = 1.4
C1 = float(np.float32(POS_RADIUS**2 - EPS))  # pos: pd2 < C1
C2 = float(np.float32(NEG_RADIUS**2 - EPS))  # neg: pd2 > C2
V = 1.0e13  # threshold-fold scale; fp32 ulp at V*C2 magnitude >> max fd2
DSHIFT = float(np.float32(V) * np.float32(C2) - np.float32(V) * np.float32(C1))
BIGF = float(np.float32(1e30))

_PROGRAM_CACHE: dict = {}
_DVE_OPS_CACHE: dict = {}


def _get_custom_ops():
    """Register the two fused select+reduce DVE ops (client-side append to
    concourse.dve_ops.OPS; row map + per-NEFF uop table stay consistent
    because both emission and table-gen read the same patched maps).

    POS: out = min(in0, in1);        accum = max(s0, rowmax(out))
    NEG: out = max(in0 + s1, in1);   accum = min(s0, rowmin(out))
    """
    if _DVE_OPS_CACHE:
        return _DVE_OPS_CACHE["pos"], _DVE_OPS_CACHE["neg"]
    import concourse.dve_ops as D
    from concourse.dve_spec import (
        C0, C1 as SC1, Spec, Src0, Src1, _has_src1, lower, maxx, minn,
    )
    from concourse.dve_uop import DveOpSpec

    def _b(x):
        return (np.asarray(x, np.float32).reshape(-1, 1)
                if np.ndim(x) else np.float32(x))

    def _ref_pos(in0, in1, c0, c1, c2):
        p = np.asarray(in0, np.float32)
        body = np.minimum(p, np.asarray(in1, np.float32).reshape(p.shape))
        b2 = body.reshape(body.shape[0], -1)
        acc = np.maximum(_b(c0), b2.max(axis=1, keepdims=True))
        return body, acc

    def _ref_neg(in0, in1, c0, c1, c2):
        p = np.asarray(in0, np.float32)
        body = np.maximum(p + _b(c1),
                          np.asarray(in1, np.float32).reshape(p.shape))
        b2 = body.reshape(body.shape[0], -1)
        acc = np.minimum(_b(c0), b2.min(axis=1, keepdims=True))
        return body, acc

    specs = {
        "HCL_SELMIN_RMAX": Spec(body=minn(Src0, Src1), accum=maxx,
                                accum_init=C0, reference=_ref_pos),
        "HCL_SELMAXS_RMIN": Spec(body=maxx(Src0 + SC1, Src1), accum=minn,
                                 accum_init=C0, reference=_ref_neg),
    }
    made = {}
    for name, spec in specs.items():
        existing = next((op for op in D.OPS if op.name == name), None)
        if existing is not None:
            made[name] = existing
            continue
        row = D._CUSTOM_DVE_ROW_BASE + len(D.OPS)
        D._SUB_OPCODE_FOR_NAME[name] = row
        shas = {}
        for ver in ("v3", "v4"):
            try:
                s = DveOpSpec(name=name, opcode=row,
                              uops=lower(spec, ver=ver),
                              rd1_en=_has_src1(spec))
                shas[ver] = s.sha(ver)
            except Exception:
                pass
        op = D.DveOp(name, spec, subdim=False, uops_sha=shas)
        D.OPS.append(op)
        D.CUSTOM_DVE_SPECS[name] = spec
        made[name] = op
    _DVE_OPS_CACHE["pos"] = made["HCL_SELMIN_RMAX"]
    _DVE_OPS_CACHE["neg"] = made["HCL_SELMAXS_RMIN"]
    return _DVE_OPS_CACHE["pos"], _DVE_OPS_CACHE["neg"]


def build_program(repeat: int = 1):
    """Build the Bass program (one NeuronCore, run SPMD on 8)."""
    import concourse.bacc as bacc
    import concourse.mybir as mybir
    import concourse.tile as tile

    pos_op, neg_op = _get_custom_ops()

    f32 = mybir.dt.float32
    f32r = mybir.dt.float32r
    bf16 = mybir.dt.bfloat16
    A = mybir.AluOpType
    AF = mybir.ActivationFunctionType
    X = mybir.AxisListType.X

    nc = bacc.Bacc("TRN2", target_bir_lowering=False, debug=False,
                   num_devices=N_CORES)
    srcF_d = nc.dram_tensor("srcF", [32, ROWS_PER_CORE], bf16,
                            kind="ExternalInput").ap()
    srcP_d = nc.dram_tensor("srcP", [KP, ROWS_PER_CORE], f32,
                            kind="ExternalInput").ap()
    tgtF_d = nc.dram_tensor("tgtF", [32, N_SEL], bf16,
                            kind="ExternalInput").ap()
    tgtP_d = nc.dram_tensor("tgtP", [KP, N_SEL], f32,
                            kind="ExternalInput").ap()
    cst_d = nc.dram_tensor("cst", [43, 4], f32,
                           kind="ExternalInput").ap()
    out_d = nc.dram_tensor("out", [2, 1], f32, kind="ExternalOutput").ap()

    with tile.TileContext(nc) as tc:
        with (
            tc.tile_pool(name="big", bufs=1) as big,
            tc.tile_pool(name="fsb", bufs=3) as fsb_p,
            tc.tile_pool(name="sq", bufs=3) as sq_p,
            tc.tile_pool(name="small", bufs=4) as small,
            tc.tile_pool(name="pf", bufs=2, space="PSUM") as pf_p,
            tc.tile_pool(name="pp1", bufs=2, space="PSUM") as pp1_p,
        ):
            rhsF = big.tile([KF, N_SEL], bf16, tag="rhsF")
            rhsP = big.tile([KP, N_SEL], f32r, tag="rhsP")
            lhsF = big.tile([KF, ROWS_PER_CORE], bf16, tag="lhsF")
            lhsP = big.tile([KP, ROWS_PER_CORE], f32r, tag="lhsP")
            prot = big.tile([3, ROWS_PER_CORE], f32, tag="prot")
            sqp = big.tile([3, ROWS_PER_CORE], f32r, tag="sqp")
            sqf = big.tile([32, ROWS_PER_CORE], f32r, tag="sqf")
            nlW = big.tile([43, 4], f32r, tag="nlW")
            ones128 = big.tile([128, 1], f32, tag="ones128")
            fp2all = big.tile([128, M_TILES], f32, tag="fp2all")
            cn2all = big.tile([128, M_TILES], f32, tag="cn2all")
            fpacc = big.tile([128, M_TILES * GN_TILES], f32, tag="fpacc")
            cnacc = big.tile([128, M_TILES * GN_TILES], f32, tag="cnacc")
            scrP = big.tile([128, GNT], f32, tag="scrP")
            scrN = big.tile([128, GNT], f32, tag="scrN")
            accT = big.tile([128, 2], f32, tag="accT")
            beps = big.tile([128, 1], f32, tag="beps")
            bpos = big.tile([128, 1], f32, tag="bpos")
            bneg = big.tile([128, 1], f32, tag="bneg")

            # src-side + const DMAs first (small; gate the critical chain)
            nc.sync.dma_start(nlW[:], cst_d.bitcast(f32r)[:])
            nc.sync.dma_start(lhsP[:], srcP_d.bitcast(f32r)[:])
            nc.sync.dma_start(lhsF[0:32, :], srcF_d[:])
            rtt_sb = big.tile([3, 4], f32r, tag="rtt")
            nc.sync.dma_start(rtt_sb[:], cst_d.bitcast(f32r)[40:43, :])
            praw = lhsP[0:3, :]
            ones_bf = big.tile([1, ROWS_PER_CORE], bf16, tag="ones_bf")
            nc.gpsimd.memset(ones_bf[:], 1.0)
            nc.sync.dma_start(lhsF[33:34, :], ones_bf[0:1, :])
            nc.gpsimd.memset(rhsF[32:33, :], 1.0)
            # preload the sqrt table set (Square lives in every set, so
            # this avoids a second 1.3us ACT_TABLE_LOAD in the tail)
            sqwarm = small.tile([1, 1], f32, tag="sqwarm")
            nc.scalar.activation(sqwarm[:], ones_bf[0:1, 0:1], AF.Sqrt)
            nc.gpsimd.memset(ones128[:], 1.0)
            nc.gpsimd.memset(beps[:], EPS)
            nc.gpsimd.memset(bpos[:], -POS_THRESH)
            nc.gpsimd.memset(bneg[:], NEG_THRESH)

            def tgt_dma(ch):
                sl = slice(ch * NT, (ch + 1) * NT)
                nc.sync.dma_start(rhsF[0:32, sl], tgtF_d[:, sl])
                nc.sync.dma_start(rhsP[:, sl], tgtP_d.bitcast(f32r)[:, sl])

            def tgt_dma2(cp):
                tgt_dma(2 * cp)
                tgt_dma(2 * cp + 1)

            # ---- src-side prep ----
            # pts chain (critical: gates the first psp1): rotate, square,
            # norm-matmul, land lhsP rows 0:4
            for ch in range(ROWS_PER_CORE // NT):
                sl = slice(ch * NT, (ch + 1) * NT)
                psrt = pf_p.tile([128, GNT], f32, tag="psf")
                psr = psrt[0:3, 0:NT]
                nc.tensor.matmul(out=psr, lhsT=rtt_sb[0:3, 0:3],
                                 rhs=praw[:, sl], start=True, stop=True)
                # 2V * (R p + t)
                nc.vector.tensor_scalar(
                    out=prot[:, sl], in0=psr,
                    scalar1=rtt_sb.bitcast(f32)[0:3, 3:4], scalar2=2.0 * V,
                    op0=A.add, op1=A.mult)
            nc.sync.dma_start(lhsP[0:3, :], prot.bitcast(f32r)[:])
            nc.vector.tensor_tensor(out=sqp[:], in0=prot[:], in1=prot[:],
                                    op=A.mult)
            for ch in range(ROWS_PER_CORE // NT):
                sl = slice(ch * NT, (ch + 1) * NT)
                psnt = pf_p.tile([128, GNT], f32, tag="psf")
                psn = psnt[0:1, 0:NT]
                nc.tensor.matmul(out=psn, lhsT=nlW[0:3, 1:2],
                                 rhs=sqp[:, sl], start=True, stop=True)
                stg = small.tile([1, NT], f32, tag="stg")
                nc.vector.tensor_copy(stg[:], psn)
                nc.sync.dma_start(lhsP[3:4, sl], stg.bitcast(f32r)[0:1, :])
            # feats norm row |a|^2 (bf16, from (-2a)^2 * 0.25)
            nc.vector.tensor_tensor(out=sqf[:], in0=lhsF[0:32, :],
                                    in1=lhsF[0:32, :], op=A.mult)
            for ch in range(ROWS_PER_CORE // NT):
                sl = slice(ch * NT, (ch + 1) * NT)
                psnt = pf_p.tile([128, GNT], f32, tag="psf")
                psn = psnt[0:1, 0:NT]
                nc.tensor.matmul(out=psn, lhsT=nlW[0:32, 0:1],
                                 rhs=sqf[0:32, sl], start=True, stop=True)
                stgb = small.tile([1, NT], bf16, tag="stgb")
                nc.vector.tensor_copy(stgb[:], psn)
                nc.sync.dma_start(lhsF[32:33, sl], stgb[0:1, :])

            # ---- tgt-side prep compute for one 512 chunk: squares of b
            # and q -> one K=35 norm matmul -> land rhsF row 32 (bf16) and
            # rhsP row 4 (f32r).  Split into two halves, spread through
            # the main loop so the Act queue never hiccups much ----
            def tgt_sq(ch):
                sl = slice(ch * NT, (ch + 1) * NT)
                sq = sq_p.tile([35, NT], f32r, tag="sqt")
                nc.scalar.activation(sq[0:32, :], rhsF[0:32, sl], AF.Square)
                nc.scalar.activation(sq[32:35, :], rhsP.bitcast(f32)[0:3, sl],
                                     AF.Square)
                return sq

            def tgt_norm(ch, sq):
                sl = slice(ch * NT, (ch + 1) * NT)
                psnt = pf_p.tile([128, GNT], f32, tag="psf")
                psnB = psnt[0:1, 0:NT]
                psnQ = psnt[0:1, NT:GNT]
                nc.tensor.matmul(out=psnB, lhsT=nlW[0:35, 2:3],
                                 rhs=sq[0:35, :], start=True, stop=True)
                nc.tensor.matmul(out=psnQ, lhsT=nlW[0:35, 3:4],
                                 rhs=sq[0:35, :], start=True, stop=True)
                stgb = small.tile([1, NT], bf16, tag="stgb2")
                nc.scalar.copy(stgb[:], psnB)
                nc.sync.dma_start(rhsF[33:34, sl], stgb[0:1, :])
                stg = small.tile([1, NT], f32, tag="stg2")
                nc.scalar.copy(stg[:], psnQ)
                nc.sync.dma_start(rhsP[4:5, sl], stg.bitcast(f32r)[0:1, :])

            for cp in range(2):
                tgt_dma2(cp)
            for ch in range(4):
                tgt_norm(ch, tgt_sq(ch))

            def main_loop(_iv=None):
                for n in range(GN_TILES):
                    for m in range(M_TILES):
                        if repeat == 1 and n < 6:
                            if m == 1:
                                tgt_dma2(n + 2)
                            elif m == 2:
                                tgt_norm(2 * n + 4, tgt_sq(2 * n + 4))
                            elif m == 5:
                                tgt_norm(2 * n + 5, tgt_sq(2 * n + 5))
                        msl = slice(m * 128, (m + 1) * 128)
                        psf = pf_p.tile([128, GNT], f32, tag="psf")
                        psp1 = pp1_p.tile([128, GNT], f32, tag="psp1")
                        for g in range(2):
                            nsl = slice(n * GNT + g * NT,
                                        n * GNT + (g + 1) * NT)
                            gsl = slice(g * NT, (g + 1) * NT)
                            nc.tensor.matmul(out=psf[:, gsl],
                                             lhsT=lhsF[:, msl],
                                             rhs=rhsF[:, nsl],
                                             start=True, stop=True)
                        for g in range(2):
                            nsl = slice(n * GNT + g * NT,
                                        n * GNT + (g + 1) * NT)
                            gsl = slice(g * NT, (g + 1) * NT)
                            nc.tensor.matmul(out=psp1[:, gsl],
                                             lhsT=lhsP[0:6, msl],
                                             rhs=rhsP[0:6, nsl],
                                             start=True, stop=True)
                        fsb = fsb_p.tile([128, GNT], f32, tag="fsb")
                        nc.scalar.copy(fsb[:], psf[:])
                        c = m * GN_TILES + n
                        nc.vector._custom_dve(
                            pos_op, out=scrP[:], in0=psp1[:], in1=fsb[:],
                            s0=0.0, accum_out=fpacc[:, c:c + 1])
                        nc.vector._custom_dve(
                            neg_op, out=scrN[:], in0=psp1[:], in1=fsb[:],
                            s0=BIGF, s1=DSHIFT,
                            accum_out=cnacc[:, c:c + 1])
                        if n == GN_TILES - 1:
                            csl = slice(m * GN_TILES, (m + 1) * GN_TILES)
                            nc.vector.tensor_reduce(
                                out=fp2all[:, m:m + 1], in_=fpacc[:, csl],
                                op=A.max, axis=X)
                            nc.vector.tensor_reduce(
                                out=cn2all[:, m:m + 1], in_=cnacc[:, csl],
                                op=A.min, axis=X)

            if repeat == 1:
                main_loop()
            else:
                with tc.For_i(0, repeat, 1) as iv:
                    main_loop(iv)

            # ---- tail: sqrt / relu thresholds / partition sums ----
            fp = small.tile([128, M_TILES], f32, tag="fp")
            cn = small.tile([128, M_TILES], f32, tag="cn")
            nc.scalar.activation(fp[:], fp2all[:], AF.Sqrt, bias=beps[:])
            nc.scalar.activation(cn[:], cn2all[:], AF.Sqrt, bias=beps[:])
            pl = small.tile([128, M_TILES], f32, tag="pl")
            nl = small.tile([128, M_TILES], f32, tag="nl")
            nc.scalar.activation(pl[:], fp[:], AF.Relu, bias=bpos[:])
            nc.scalar.activation(nl[:], cn[:], AF.Relu, bias=bneg[:],
                                 scale=-1.0)
            nc.vector.tensor_reduce(out=accT[:, 0:1], in_=pl[:], op=A.add,
                                    axis=X)
            nc.vector.tensor_reduce(out=accT[:, 1:2], in_=nl[:], op=A.add,
                                    axis=X)
            psot = pf_p.tile([128, GNT], f32, tag="psf")
            pso = psot[0:2, 0:1]
            nc.tensor.matmul(out=pso, lhsT=accT[:], rhs=ones128[:],
                             start=True, stop=True)
            res_sb = small.tile([2, 1], f32, tag="res")
            nc.scalar.copy(res_sb[:], pso)
            nc.sync.dma_start(out_d[:], res_sb[:])

    nc.compile()
    return nc


def make_in_maps(src_pcd, tgt_pcd, src_feats, tgt_feats, correspondence,
                 rot, trans):
    """Host-side gather/shard/layout (indexing, transpose, exact constant
    scaling, dtype cast and constant fills only)."""
    import ml_dtypes
    bf16 = ml_dtypes.bfloat16
    ci = np.asarray(correspondence[:, 0]).astype(np.int64)
    cj = np.asarray(correspondence[:, 1]).astype(np.int64)
    src_pcd = np.asarray(src_pcd, np.float32)
    tgt_pcd = np.asarray(tgt_pcd, np.float32)
    src_feats = np.asarray(src_feats, np.float32)
    tgt_feats = np.asarray(tgt_feats, np.float32)

    # center pts at the box center: tf32 (fp32r) input rounding error is
    # relative to coordinate magnitude; |p-q|^2 is shift-invariant
    CEN = np.float32(0.1)

    tgtF = tgt_feats[cj].T.astype(bf16)
    # rows 32 (ones) and 33 (|b|^2) are device-side
    tgtP = np.zeros((KP, N_SEL), np.float32)
    tgtP[0:3] = tgt_pcd[cj].T - CEN
    tgtP[3] = 1.0
    # [4] = |q|^2 (device)
    tgtP[5] = np.float32(V) * np.float32(C1)

    srcF = (np.float32(-2.0) * src_feats[ci].T).astype(bf16)
    # rows 32 (|a|^2) and 33 (ones) are device-side
    srcP = np.zeros((KP, ROWS_PER_CORE * N_CORES), np.float32)
    srcP[0:3] = src_pcd[ci].T  # device applies rot/trans and 2V
    # [3] = -V|p^|^2 (device)
    srcP[4] = -np.float32(V)
    srcP[5] = 1.0



    # norm-term matmul weights:
    #   col0: |a|^2 from (-2a)^2 (K=32 over sqf)
    #   col1: -V|p^|^2 from (2Vp^)^2 (K=3 over sqp)
    #   col2: |b|^2 (rows 0:32 of sq chunk); col3: |q|^2 (rows 32:35)
    cst = np.zeros((43, 4), np.float32)
    cst[0:32, 0] = 0.25
    cst[0:3, 1] = np.float32(-1.0 / (4.0 * V))
    cst[0:32, 2] = 1.0
    cst[32:35, 3] = 1.0
    cst[40:43, 0:3] = np.asarray(rot, np.float32).T
    cst[40:43, 3] = np.asarray(trans, np.float32)[:, 0] - CEN

    in_maps = []
    for c in range(N_CORES):
        sl = slice(c * ROWS_PER_CORE, (c + 1) * ROWS_PER_CORE)
        in_maps.append({
            "srcF": np.ascontiguousarray(srcF[:, sl]),
            "srcP": np.ascontiguousarray(srcP[:, sl]),
            "tgtF": tgtF,
            "tgtP": tgtP,
            "cst": cst,
        })
    return in_maps


def combine_outputs(results):
    """Host-side unshard: sum per-core partial sums, divide by N."""
    tot = np.zeros(2, np.float32)
    for r in results:
        tot += r["out"][:, 0].astype(np.float32)
    loss = np.float32(tot[0] / np.float32(N_SEL) + tot[1] / np.float32(N_SEL))
    return np.float32(loss)


def kernel(src_pcd, tgt_pcd, src_feats, tgt_feats, correspondence, rot,
           trans):
    from concourse import bass_utils

    key = ("prog", 1)
    if key not in _PROGRAM_CACHE:
        _PROGRAM_CACHE[key] = build_program(repeat=1)
    nc = _PROGRAM_CACHE[key]
    in_maps = make_in_maps(src_pcd, tgt_pcd, src_feats, tgt_feats,
                           correspondence, rot, trans)
    res = bass_utils.run_bass_kernel_spmd(nc, in_maps,
                                          core_ids=list(range(N_CORES)))
    return combine_outputs(res.results)


# revision 37
# speedup vs baseline: 1.0036x; 1.0036x over previous
"""Trainium2 Bass kernel for nn_HardestContrastiveLoss.

Strategy (1D row-parallel cdist, per sharding hint):
  - Host: gather the selected correspondences (indexing/transpose + exact
    constant scaling + dtype cast), shard 8192 selected rows as 1024/core.
  - Device (per core, identical program, different data):
      * prep: rigid-transform gathered src points (small matmul + fused
        add/scale), square passes + ones-matmuls for the norm terms
      * two matmuls per [128, 512] tile (PE pinned at 1.2 GHz on this
        system -- HAM never un-throttles -- so matmul cost is
        streaming-bound at ~1 col/cycle):
          psf  = -2a.b + |a|^2 + |b|^2  (feats, bf16, K=34)
          psp1 = V*(C1 - |p-q|^2)       (threshold-folded pts, f32r, K=6)
        Feats ship as bf16 (halves the dominant DMA volume -- the 8 cores
        share the chip DMA engines, so input landing time is
        bytes-bound -- and enables fast-weight-load).  With V=1e13 the
        fp32 pts accumulation quantizes psp1 so every pos/neg gap is far
        larger than any feats distance^2: an elementwise min/max against
        psf is an exact mask-select.
      * per [128, 1024] macro tile (column-major (n, m) order so the tgt
        DMA/prep pipeline hides behind compute):
          Act stages psf -> fsb; DVE runs ONE fused custom op per side
          (select + free-dim reduce + seeded accum in a single pass):
            pos: accum[c] = max(0,   max_k min(psp1, fd2))
            neg: accum[c] = min(BIG, min_k max(psp1 + D, fd2))
      * tail: per-m reduces, clamp, sqrt, relu thresholds, ones-matmul
  - Host: sum the 8 per-core [2,1] partials, divide by N (the "all-reduce").

Operand layout (base partition 0 for both matmuls):
  srcF/tgtF [34, .] bf16:  0:32 = -2a (host-scaled) / b;  32 = |a|^2
    (dev) / 1;  33 = 1 / |b|^2 (dev)
  srcP/tgtP [8, .] f32r:   0:3 = 2V*(R p + t) (dev) / q;  3 = -V|p^|^2
    (dev) / 1;  4 = -V / |q|^2 (dev);  5 = 1 / V*C1
"""

import numpy as np

N_SEL = 8192
N_CORES = 8
ROWS_PER_CORE = N_SEL // N_CORES  # 1024
M_TILES = ROWS_PER_CORE // 128  # 8
NT = 512  # matmul tile (one PSUM bank)
GNT = 1024  # macro tile (two PSUM banks)
GN_TILES = N_SEL // GNT  # 8
N_CHUNKS = N_SEL // NT  # 16
KF = 34  # feats rows (-2a | norms)
KP = 8   # pts rows

EPS = 1e-7
POS_RADIUS = 0.0375
NEG_RADIUS = 0.1
POS_THRESH = 0.1
NEG_THRESH = 1.4
C1 = float(np.float32(POS_RADIUS**2 - EPS))  # pos: pd2 < C1
C2 = float(np.float32(NEG_RADIUS**2 - EPS))  # neg: pd2 > C2
V = 1.0e13  # threshold-fold scale; fp32 ulp at V*C2 magnitude >> max fd2
DSHIFT = float(np.float32(V) * np.float32(C2) - np.float32(V) * np.float32(C1))
BIGF = float(np.float32(1e30))

_PROGRAM_CACHE: dict = {}
_DVE_OPS_CACHE: dict = {}


def _get_custom_ops():
    """Register the two fused select+reduce DVE ops (client-side append to
    concourse.dve_ops.OPS; row map + per-NEFF uop table stay consistent
    because both emission and table-gen read the same patched maps).

    POS: out = min(in0, in1);        accum = max(s0, rowmax(out))
    NEG: out = max(in0 + s1, in1);   accum = min(s0, rowmin(out))
    """
    if _DVE_OPS_CACHE:
        return _DVE_OPS_CACHE["pos"], _DVE_OPS_CACHE["neg"]
    import concourse.dve_ops as D
    from concourse.dve_spec import (
        C0, C1 as SC1, Spec, Src0, Src1, _has_src1, lower, maxx, minn,
    )
    from concourse.dve_uop import DveOpSpec

    def _b(x):
        return (np.asarray(x, np.float32).reshape(-1, 1)
                if np.ndim(x) else np.float32(x))

    def _ref_pos(in0, in1, c0, c1, c2):
        p = np.asarray(in0, np.float32)
        body = np.minimum(p, np.asarray(in1, np.float32).reshape(p.shape))
        b2 = body.reshape(body.shape[0], -1)
        acc = np.maximum(_b(c0), b2.max(axis=1, keepdims=True))
        return body, acc

    def _ref_neg(in0, in1, c0, c1, c2):
        p = np.asarray(in0, np.float32)
        body = np.maximum(p + _b(c1),
                          np.asarray(in1, np.float32).reshape(p.shape))
        b2 = body.reshape(body.shape[0], -1)
        acc = np.minimum(_b(c0), b2.min(axis=1, keepdims=True))
        return body, acc

    specs = {
        "HCL_SELMIN_RMAX": Spec(body=minn(Src0, Src1), accum=maxx,
                                accum_init=C0, reference=_ref_pos),
        "HCL_SELMAXS_RMIN": Spec(body=maxx(Src0 + SC1, Src1), accum=minn,
                                 accum_init=C0, reference=_ref_neg),
    }
    made = {}
    for name, spec in specs.items():
        existing = next((op for op in D.OPS if op.name == name), None)
        if existing is not None:
            made[name] = existing
            continue
        row = D._CUSTOM_DVE_ROW_BASE + len(D.OPS)
        D._SUB_OPCODE_FOR_NAME[name] = row
        shas = {}
        for ver in ("v3", "v4"):
            try:
                s = DveOpSpec(name=name, opcode=row,
                              uops=lower(spec, ver=ver),
                              rd1_en=_has_src1(spec))
                shas[ver] = s.sha(ver)
            except Exception:
                pass
        op = D.DveOp(name, spec, subdim=False, uops_sha=shas)
        D.OPS.append(op)
        D.CUSTOM_DVE_SPECS[name] = spec
        made[name] = op
    _DVE_OPS_CACHE["pos"] = made["HCL_SELMIN_RMAX"]
    _DVE_OPS_CACHE["neg"] = made["HCL_SELMAXS_RMIN"]
    return _DVE_OPS_CACHE["pos"], _DVE_OPS_CACHE["neg"]


def build_program(repeat: int = 1):
    """Build the Bass program (one NeuronCore, run SPMD on 8)."""
    import concourse.bacc as bacc
    import concourse.mybir as mybir
    import concourse.tile as tile

    pos_op, neg_op = _get_custom_ops()

    f32 = mybir.dt.float32
    f32r = mybir.dt.float32r
    bf16 = mybir.dt.bfloat16
    A = mybir.AluOpType
    AF = mybir.ActivationFunctionType
    X = mybir.AxisListType.X

    nc = bacc.Bacc("TRN2", target_bir_lowering=False, debug=False,
                   num_devices=N_CORES)
    srcF_d = nc.dram_tensor("srcF", [32, ROWS_PER_CORE], bf16,
                            kind="ExternalInput").ap()
    srcP_d = nc.dram_tensor("srcP", [KP, ROWS_PER_CORE], f32,
                            kind="ExternalInput").ap()
    tgtF_d = nc.dram_tensor("tgtF", [32, N_SEL], bf16,
                            kind="ExternalInput").ap()
    tgtP_d = nc.dram_tensor("tgtP", [KP, N_SEL], f32,
                            kind="ExternalInput").ap()
    cst_d = nc.dram_tensor("cst", [43, 4], f32,
                           kind="ExternalInput").ap()
    out_d = nc.dram_tensor("out", [2, 1], f32, kind="ExternalOutput").ap()

    with tile.TileContext(nc) as tc:
        with (
            tc.tile_pool(name="big", bufs=1) as big,
            tc.tile_pool(name="fsb", bufs=4) as fsb_p,
            tc.tile_pool(name="sq", bufs=3) as sq_p,
            tc.tile_pool(name="small", bufs=4) as small,
            tc.tile_pool(name="pf", bufs=2, space="PSUM") as pf_p,
            tc.tile_pool(name="pp1", bufs=2, space="PSUM") as pp1_p,
        ):
            rhsF = big.tile([KF, N_SEL], bf16, tag="rhsF")
            rhsP = big.tile([KP, N_SEL], f32r, tag="rhsP")
            lhsF = big.tile([KF, ROWS_PER_CORE], bf16, tag="lhsF")
            lhsP = big.tile([KP, ROWS_PER_CORE], f32r, tag="lhsP")
            prot = big.tile([3, ROWS_PER_CORE], f32, tag="prot")
            sqp = big.tile([3, ROWS_PER_CORE], f32r, tag="sqp")
            sqf = big.tile([32, ROWS_PER_CORE], f32r, tag="sqf")
            nlW = big.tile([43, 4], f32r, tag="nlW")
            ones128 = big.tile([128, 1], f32, tag="ones128")
            fp2all = big.tile([128, M_TILES], f32, tag="fp2all")
            cn2all = big.tile([128, M_TILES], f32, tag="cn2all")
            fpacc = big.tile([128, M_TILES * GN_TILES], f32, tag="fpacc")
            cnacc = big.tile([128, M_TILES * GN_TILES], f32, tag="cnacc")
            scrP = big.tile([128, GNT], f32, tag="scrP")
            scrN = big.tile([128, GNT], f32, tag="scrN")
            accT = big.tile([128, 2], f32, tag="accT")
            beps = big.tile([128, 1], f32, tag="beps")
            bpos = big.tile([128, 1], f32, tag="bpos")
            bneg = big.tile([128, 1], f32, tag="bneg")

            # src-side + const DMAs first (small; gate the critical chain)
            nc.sync.dma_start(nlW[:], cst_d.bitcast(f32r)[:])
            nc.sync.dma_start(lhsP[:], srcP_d.bitcast(f32r)[:])
            nc.sync.dma_start(lhsF[0:32, :], srcF_d[:])
            rtt_sb = big.tile([3, 4], f32r, tag="rtt")
            nc.sync.dma_start(rtt_sb[:], cst_d.bitcast(f32r)[40:43, :])
            praw = lhsP[0:3, :]
            ones_bf = big.tile([1, ROWS_PER_CORE], bf16, tag="ones_bf")
            nc.gpsimd.memset(ones_bf[:], 1.0)
            nc.sync.dma_start(lhsF[33:34, :], ones_bf[0:1, :])
            nc.gpsimd.memset(rhsF[32:33, :], 1.0)
            # preload the sqrt table set (Square lives in every set, so
            # this avoids a second 1.3us ACT_TABLE_LOAD in the tail)
            sqwarm = small.tile([1, 1], f32, tag="sqwarm")
            nc.scalar.activation(sqwarm[:], ones_bf[0:1, 0:1], AF.Sqrt)
            nc.gpsimd.memset(ones128[:], 1.0)
            nc.gpsimd.memset(beps[:], EPS)
            nc.gpsimd.memset(bpos[:], -POS_THRESH)
            nc.gpsimd.memset(bneg[:], NEG_THRESH)

            def tgt_dma(ch):
                sl = slice(ch * NT, (ch + 1) * NT)
                nc.sync.dma_start(rhsF[0:32, sl], tgtF_d[:, sl])
                nc.sync.dma_start(rhsP[:, sl], tgtP_d.bitcast(f32r)[:, sl])

            def tgt_dma2(cp):
                tgt_dma(2 * cp)
                tgt_dma(2 * cp + 1)

            # ---- src-side prep ----
            # pts chain (critical: gates the first psp1): rotate, square,
            # norm-matmul, land lhsP rows 0:4
            for ch in range(ROWS_PER_CORE // NT):
                sl = slice(ch * NT, (ch + 1) * NT)
                psrt = pf_p.tile([128, GNT], f32, tag="psf")
                psr = psrt[0:3, 0:NT]
                nc.tensor.matmul(out=psr, lhsT=rtt_sb[0:3, 0:3],
                                 rhs=praw[:, sl], start=True, stop=True)
                # 2V * (R p + t)
                nc.vector.tensor_scalar(
                    out=prot[:, sl], in0=psr,
                    scalar1=rtt_sb.bitcast(f32)[0:3, 3:4], scalar2=2.0 * V,
                    op0=A.add, op1=A.mult)
            nc.sync.dma_start(lhsP[0:3, :], prot.bitcast(f32r)[:])
            nc.vector.tensor_tensor(out=sqp[:], in0=prot[:], in1=prot[:],
                                    op=A.mult)
            for ch in range(ROWS_PER_CORE // NT):
                sl = slice(ch * NT, (ch + 1) * NT)
                psnt = pf_p.tile([128, GNT], f32, tag="psf")
                psn = psnt[0:1, 0:NT]
                nc.tensor.matmul(out=psn, lhsT=nlW[0:3, 1:2],
                                 rhs=sqp[:, sl], start=True, stop=True)
                stg = small.tile([1, NT], f32, tag="stg")
                nc.vector.tensor_copy(stg[:], psn)
                nc.sync.dma_start(lhsP[3:4, sl], stg.bitcast(f32r)[0:1, :])
            # feats norm row |a|^2 (bf16, from (-2a)^2 * 0.25)
            nc.vector.tensor_tensor(out=sqf[:], in0=lhsF[0:32, :],
                                    in1=lhsF[0:32, :], op=A.mult)
            for ch in range(ROWS_PER_CORE // NT):
                sl = slice(ch * NT, (ch + 1) * NT)
                psnt = pf_p.tile([128, GNT], f32, tag="psf")
                psn = psnt[0:1, 0:NT]
                nc.tensor.matmul(out=psn, lhsT=nlW[0:32, 0:1],
                                 rhs=sqf[0:32, sl], start=True, stop=True)
                stgb = small.tile([1, NT], bf16, tag="stgb")
                nc.vector.tensor_copy(stgb[:], psn)
                nc.sync.dma_start(lhsF[32:33, sl], stgb[0:1, :])

            # ---- tgt-side prep compute for one 512 chunk: squares of b
            # and q -> one K=35 norm matmul -> land rhsF row 32 (bf16) and
            # rhsP row 4 (f32r).  Split into two halves, spread through
            # the main loop so the Act queue never hiccups much ----
            def tgt_sqF(ch):
                sl = slice(ch * NT, (ch + 1) * NT)
                sq = sq_p.tile([35, NT], f32r, tag="sqt")
                nc.scalar.activation(sq[0:32, :], rhsF[0:32, sl], AF.Square)
                return sq

            def tgt_sqP(ch, sq):
                sl = slice(ch * NT, (ch + 1) * NT)
                nc.scalar.activation(sq[32:35, :], rhsP.bitcast(f32)[0:3, sl],
                                     AF.Square)
                return sq

            def tgt_sq(ch):
                return tgt_sqP(ch, tgt_sqF(ch))

            def tgt_norm(ch, sq):
                sl = slice(ch * NT, (ch + 1) * NT)
                psnt = pf_p.tile([128, GNT], f32, tag="psf")
                psnB = psnt[0:1, 0:NT]
                psnQ = psnt[0:1, NT:GNT]
                nc.tensor.matmul(out=psnB, lhsT=nlW[0:35, 2:3],
                                 rhs=sq[0:35, :], start=True, stop=True)
                nc.tensor.matmul(out=psnQ, lhsT=nlW[0:35, 3:4],
                                 rhs=sq[0:35, :], start=True, stop=True)
                stgb = small.tile([1, NT], bf16, tag="stgb2")
                nc.scalar.copy(stgb[:], psnB)
                nc.sync.dma_start(rhsF[33:34, sl], stgb[0:1, :])
                stg = small.tile([1, NT], f32, tag="stg2")
                nc.scalar.copy(stg[:], psnQ)
                nc.sync.dma_start(rhsP[4:5, sl], stg.bitcast(f32r)[0:1, :])

            for cp in range(2):
                tgt_dma2(cp)
            for ch in range(4):
                tgt_norm(ch, tgt_sq(ch))

            sq_pend = [None, None]

            def main_loop(_iv=None):
                for n in range(GN_TILES):
                    for m in range(M_TILES):
                        if repeat == 1 and n < 6:
                            if m == 0:
                                tgt_dma2(n + 2)
                            elif m == 1:
                                sq_pend[0] = tgt_sqF(2 * n + 4)
                            elif m == 2:
                                tgt_sqP(2 * n + 4, sq_pend[0])
                            elif m == 3:
                                tgt_norm(2 * n + 4, sq_pend[0])
                            elif m == 4:
                                sq_pend[1] = tgt_sqF(2 * n + 5)
                            elif m == 5:
                                tgt_sqP(2 * n + 5, sq_pend[1])
                            elif m == 6:
                                tgt_norm(2 * n + 5, sq_pend[1])
                        msl = slice(m * 128, (m + 1) * 128)
                        psf = pf_p.tile([128, GNT], f32, tag="psf")
                        psp1 = pp1_p.tile([128, GNT], f32, tag="psp1")
                        for g in range(2):
                            nsl = slice(n * GNT + g * NT,
                                        n * GNT + (g + 1) * NT)
                            gsl = slice(g * NT, (g + 1) * NT)
                            nc.tensor.matmul(out=psf[:, gsl],
                                             lhsT=lhsF[:, msl],
                                             rhs=rhsF[:, nsl],
                                             start=True, stop=True)
                        for g in range(2):
                            nsl = slice(n * GNT + g * NT,
                                        n * GNT + (g + 1) * NT)
                            gsl = slice(g * NT, (g + 1) * NT)
                            nc.tensor.matmul(out=psp1[:, gsl],
                                             lhsT=lhsP[0:6, msl],
                                             rhs=rhsP[0:6, nsl],
                                             start=True, stop=True)
                        fsb = fsb_p.tile([128, GNT], f32, tag="fsb")
                        nc.scalar.copy(fsb[:], psf[:])
                        c = m * GN_TILES + n
                        nc.vector._custom_dve(
                            pos_op, out=scrP[:], in0=psp1[:], in1=fsb[:],
                            s0=0.0, accum_out=fpacc[:, c:c + 1])
                        nc.vector._custom_dve(
                            neg_op, out=scrN[:], in0=psp1[:], in1=fsb[:],
                            s0=BIGF, s1=DSHIFT,
                            accum_out=cnacc[:, c:c + 1])
                        if n == GN_TILES - 1:
                            csl = slice(m * GN_TILES, (m + 1) * GN_TILES)
                            nc.vector.tensor_reduce(
                                out=fp2all[:, m:m + 1], in_=fpacc[:, csl],
                                op=A.max, axis=X)
                            nc.vector.tensor_reduce(
                                out=cn2all[:, m:m + 1], in_=cnacc[:, csl],
                                op=A.min, axis=X)

            if repeat == 1:
                main_loop()
            else:
                with tc.For_i(0, repeat, 1) as iv:
                    main_loop(iv)

            # ---- tail: sqrt / relu thresholds / partition sums ----
            fp = small.tile([128, M_TILES], f32, tag="fp")
            cn = small.tile([128, M_TILES], f32, tag="cn")
            nc.scalar.activation(fp[:], fp2all[:], AF.Sqrt, bias=beps[:])
            nc.scalar.activation(cn[:], cn2all[:], AF.Sqrt, bias=beps[:])
            pl = small.tile([128, M_TILES], f32, tag="pl")
            nl = small.tile([128, M_TILES], f32, tag="nl")
            nc.scalar.activation(pl[:], fp[:], AF.Relu, bias=bpos[:])
            nc.scalar.activation(nl[:], cn[:], AF.Relu, bias=bneg[:],
                                 scale=-1.0)
            nc.vector.tensor_reduce(out=accT[:, 0:1], in_=pl[:], op=A.add,
                                    axis=X)
            nc.vector.tensor_reduce(out=accT[:, 1:2], in_=nl[:], op=A.add,
                                    axis=X)
            psot = pf_p.tile([128, GNT], f32, tag="psf")
            pso = psot[0:2, 0:1]
            nc.tensor.matmul(out=pso, lhsT=accT[:], rhs=ones128[:],
                             start=True, stop=True)
            res_sb = small.tile([2, 1], f32, tag="res")
            nc.scalar.copy(res_sb[:], pso)
            nc.sync.dma_start(out_d[:], res_sb[:])

    nc.compile()
    return nc


def make_in_maps(src_pcd, tgt_pcd, src_feats, tgt_feats, correspondence,
                 rot, trans):
    """Host-side gather/shard/layout (indexing, transpose, exact constant
    scaling, dtype cast and constant fills only)."""
    import ml_dtypes
    bf16 = ml_dtypes.bfloat16
    ci = np.asarray(correspondence[:, 0]).astype(np.int64)
    cj = np.asarray(correspondence[:, 1]).astype(np.int64)
    src_pcd = np.asarray(src_pcd, np.float32)
    tgt_pcd = np.asarray(tgt_pcd, np.float32)
    src_feats = np.asarray(src_feats, np.float32)
    tgt_feats = np.asarray(tgt_feats, np.float32)

    # center pts at the box center: tf32 (fp32r) input rounding error is
    # relative to coordinate magnitude; |p-q|^2 is shift-invariant
    CEN = np.float32(0.1)

    tgtF = tgt_feats[cj].T.astype(bf16)
    # rows 32 (ones) and 33 (|b|^2) are device-side
    tgtP = np.zeros((KP, N_SEL), np.float32)
    tgtP[0:3] = tgt_pcd[cj].T - CEN
    tgtP[3] = 1.0
    # [4] = |q|^2 (device)
    tgtP[5] = np.float32(V) * np.float32(C1)

    srcF = (np.float32(-2.0) * src_feats[ci].T).astype(bf16)
    # rows 32 (|a|^2) and 33 (ones) are device-side
    srcP = np.zeros((KP, ROWS_PER_CORE * N_CORES), np.float32)
    srcP[0:3] = src_pcd[ci].T  # device applies rot/trans and 2V
    # [3] = -V|p^|^2 (device)
    srcP[4] = -np.float32(V)
    srcP[5] = 1.0



    # norm-term matmul weights:
    #   col0: |a|^2 from (-2a)^2 (K=32 over sqf)
    #   col1: -V|p^|^2 from (2Vp^)^2 (K=3 over sqp)
    #   col2: |b|^2 (rows 0:32 of sq chunk); col3: |q|^2 (rows 32:35)
    cst = np.zeros((43, 4), np.float32)
    cst[0:32, 0] = 0.25
    cst[0:3, 1] = np.float32(-1.0 / (4.0 * V))
    cst[0:32, 2] = 1.0
    cst[32:35, 3] = 1.0
    cst[40:43, 0:3] = np.asarray(rot, np.float32).T
    cst[40:43, 3] = np.asarray(trans, np.float32)[:, 0] - CEN

    in_maps = []
    for c in range(N_CORES):
        sl = slice(c * ROWS_PER_CORE, (c + 1) * ROWS_PER_CORE)
        in_maps.append({
            "srcF": np.ascontiguousarray(srcF[:, sl]),
            "srcP": np.ascontiguousarray(srcP[:, sl]),
            "tgtF": tgtF,
            "tgtP": tgtP,
            "cst": cst,
        })
    return in_maps


def combine_outputs(results):
    """Host-side unshard: sum per-core partial sums, divide by N."""
    tot = np.zeros(2, np.float32)
    for r in results:
        tot += r["out"][:, 0].astype(np.float32)
    loss = np.float32(tot[0] / np.float32(N_SEL) + tot[1] / np.float32(N_SEL))
    return np.float32(loss)


def kernel(src_pcd, tgt_pcd, src_feats, tgt_feats, correspondence, rot,
           trans):
    from concourse import bass_utils

    key = ("prog", 1)
    if key not in _PROGRAM_CACHE:
        _PROGRAM_CACHE[key] = build_program(repeat=1)
    nc = _PROGRAM_CACHE[key]
    in_maps = make_in_maps(src_pcd, tgt_pcd, src_feats, tgt_feats,
                           correspondence, rot, trans)
    res = bass_utils.run_bass_kernel_spmd(nc, in_maps,
                                          core_ids=list(range(N_CORES)))
    return combine_outputs(res.results)


# revision 38
# speedup vs baseline: 1.0287x; 1.0250x over previous
"""Trainium2 Bass kernel for nn_HardestContrastiveLoss.

Strategy (1D row-parallel cdist, per sharding hint):
  - Host: gather the selected correspondences (indexing/transpose + exact
    constant scaling + dtype cast), shard 8192 selected rows as 1024/core.
  - Device (per core, identical program, different data):
      * prep: rigid-transform gathered src points (small matmul + fused
        add/scale), square passes + ones-matmuls for the norm terms
      * two matmuls per [128, 512] tile (PE pinned at 1.2 GHz on this
        system -- HAM never un-throttles -- so matmul cost is
        streaming-bound at ~1 col/cycle):
          psf  = -2a.b + |a|^2 + |b|^2  (feats, bf16, K=34)
          psp1 = V*(C1 - |p-q|^2)       (threshold-folded pts, f32r, K=6)
        Feats ship as bf16 (halves the dominant DMA volume -- the 8 cores
        share the chip DMA engines, so input landing time is
        bytes-bound -- and enables fast-weight-load).  With V=1e13 the
        fp32 pts accumulation quantizes psp1 so every pos/neg gap is far
        larger than any feats distance^2: an elementwise min/max against
        psf is an exact mask-select.
      * per [128, 1024] macro tile (column-major (n, m) order so the tgt
        DMA/prep pipeline hides behind compute):
          Act stages psf -> fsb; DVE runs ONE fused custom op per side
          (select + free-dim reduce + seeded accum in a single pass):
            pos: accum[c] = max(0,   max_k min(psp1, fd2))
            neg: accum[c] = min(BIG, min_k max(psp1 + D, fd2))
      * tail: per-m reduces, clamp, sqrt, relu thresholds, ones-matmul
  - Host: sum the 8 per-core [2,1] partials, divide by N (the "all-reduce").

Operand layout (base partition 0 for both matmuls):
  srcF/tgtF [34, .] bf16:  0:32 = -2a (host-scaled) / b;  32 = |a|^2
    (dev) / 1;  33 = 1 / |b|^2 (dev)
  srcP/tgtP [8, .] f32r:   0:3 = 2V*(R p + t) (dev) / q;  3 = -V|p^|^2
    (dev) / 1;  4 = -V / |q|^2 (dev);  5 = 1 / V*C1
"""

import numpy as np

N_SEL = 8192
N_CORES = 8
ROWS_PER_CORE = N_SEL // N_CORES  # 1024
M_TILES = ROWS_PER_CORE // 128  # 8
NT = 512  # matmul tile (one PSUM bank)
GNT = 1024  # macro tile (two PSUM banks)
GN_TILES = N_SEL // GNT  # 8
N_CHUNKS = N_SEL // NT  # 16
KF = 34  # feats rows (-2a | norms)
KP = 8   # pts rows

EPS = 1e-7
POS_RADIUS = 0.0375
NEG_RADIUS = 0.1
POS_THRESH = 0.1
NEG_THRESH = 1.4
C1 = float(np.float32(POS_RADIUS**2 - EPS))  # pos: pd2 < C1
C2 = float(np.float32(NEG_RADIUS**2 - EPS))  # neg: pd2 > C2
V = 1.0e13  # threshold-fold scale; fp32 ulp at V*C2 magnitude >> max fd2
DSHIFT = float(np.float32(V) * np.float32(C2) - np.float32(V) * np.float32(C1))
BIGF = float(np.float32(1e30))

_PROGRAM_CACHE: dict = {}
_DVE_OPS_CACHE: dict = {}


def _get_custom_ops():
    """Register the two fused select+reduce DVE ops (client-side append to
    concourse.dve_ops.OPS; row map + per-NEFF uop table stay consistent
    because both emission and table-gen read the same patched maps).

    POS: out = min(in0, in1);        accum = max(s0, rowmax(out))
    NEG: out = max(in0 + s1, in1);   accum = min(s0, rowmin(out))
    """
    if _DVE_OPS_CACHE:
        return _DVE_OPS_CACHE["pos"], _DVE_OPS_CACHE["neg"]
    import concourse.dve_ops as D
    from concourse.dve_spec import (
        C0, C1 as SC1, Spec, Src0, Src1, _has_src1, lower, maxx, minn,
    )
    from concourse.dve_uop import DveOpSpec

    def _b(x):
        return (np.asarray(x, np.float32).reshape(-1, 1)
                if np.ndim(x) else np.float32(x))

    def _ref_pos(in0, in1, c0, c1, c2):
        p = np.asarray(in0, np.float32)
        body = np.minimum(p, np.asarray(in1, np.float32).reshape(p.shape))
        b2 = body.reshape(body.shape[0], -1)
        acc = np.maximum(_b(c0), b2.max(axis=1, keepdims=True))
        return body, acc

    def _ref_neg(in0, in1, c0, c1, c2):
        p = np.asarray(in0, np.float32)
        body = np.maximum(p + _b(c1),
                          np.asarray(in1, np.float32).reshape(p.shape))
        b2 = body.reshape(body.shape[0], -1)
        acc = np.minimum(_b(c0), b2.min(axis=1, keepdims=True))
        return body, acc

    specs = {
        "HCL_SELMIN_RMAX": Spec(body=minn(Src0, Src1), accum=maxx,
                                accum_init=C0, reference=_ref_pos),
        "HCL_SELMAXS_RMIN": Spec(body=maxx(Src0 + SC1, Src1), accum=minn,
                                 accum_init=C0, reference=_ref_neg),
    }
    made = {}
    for name, spec in specs.items():
        existing = next((op for op in D.OPS if op.name == name), None)
        if existing is not None:
            made[name] = existing
            continue
        row = D._CUSTOM_DVE_ROW_BASE + len(D.OPS)
        D._SUB_OPCODE_FOR_NAME[name] = row
        shas = {}
        for ver in ("v3", "v4"):
            try:
                s = DveOpSpec(name=name, opcode=row,
                              uops=lower(spec, ver=ver),
                              rd1_en=_has_src1(spec))
                shas[ver] = s.sha(ver)
            except Exception:
                pass
        op = D.DveOp(name, spec, subdim=False, uops_sha=shas)
        D.OPS.append(op)
        D.CUSTOM_DVE_SPECS[name] = spec
        made[name] = op
    _DVE_OPS_CACHE["pos"] = made["HCL_SELMIN_RMAX"]
    _DVE_OPS_CACHE["neg"] = made["HCL_SELMAXS_RMIN"]
    return _DVE_OPS_CACHE["pos"], _DVE_OPS_CACHE["neg"]


def build_program(repeat: int = 1):
    """Build the Bass program (one NeuronCore, run SPMD on 8)."""
    import concourse.bacc as bacc
    import concourse.mybir as mybir
    import concourse.tile as tile

    pos_op, neg_op = _get_custom_ops()

    f32 = mybir.dt.float32
    f32r = mybir.dt.float32r
    bf16 = mybir.dt.bfloat16
    A = mybir.AluOpType
    AF = mybir.ActivationFunctionType
    X = mybir.AxisListType.X

    nc = bacc.Bacc("TRN2", target_bir_lowering=False, debug=False,
                   num_devices=N_CORES)
    srcF_d = nc.dram_tensor("srcF", [32, ROWS_PER_CORE], bf16,
                            kind="ExternalInput").ap()
    srcP_d = nc.dram_tensor("srcP", [KP, ROWS_PER_CORE], f32,
                            kind="ExternalInput").ap()
    tgtF_d = nc.dram_tensor("tgtF", [32, N_SEL], bf16,
                            kind="ExternalInput").ap()
    tgtP_d = nc.dram_tensor("tgtP", [KP, N_SEL], f32,
                            kind="ExternalInput").ap()
    cst_d = nc.dram_tensor("cst", [43, 4], f32,
                           kind="ExternalInput").ap()
    out_d = nc.dram_tensor("out", [2, 1], f32, kind="ExternalOutput").ap()

    with tile.TileContext(nc) as tc:
        with (
            tc.tile_pool(name="big", bufs=1) as big,
            tc.tile_pool(name="fsb", bufs=4) as fsb_p,
            tc.tile_pool(name="sq", bufs=3) as sq_p,
            tc.tile_pool(name="small", bufs=4) as small,
            tc.tile_pool(name="pf", bufs=2, space="PSUM") as pf_p,
            tc.tile_pool(name="pp1", bufs=2, space="PSUM") as pp1_p,
        ):
            rhsF = big.tile([KF, N_SEL], bf16, tag="rhsF")
            rhsP = big.tile([KP, N_SEL], f32r, tag="rhsP")
            lhsF = big.tile([KF, ROWS_PER_CORE], bf16, tag="lhsF")
            lhsP = big.tile([KP, ROWS_PER_CORE], f32r, tag="lhsP")
            prot = big.tile([3, ROWS_PER_CORE], f32, tag="prot")
            sqp = big.tile([3, ROWS_PER_CORE], f32r, tag="sqp")
            sqf = big.tile([32, ROWS_PER_CORE], f32r, tag="sqf")
            nlW = big.tile([43, 4], f32r, tag="nlW")
            ones128 = big.tile([128, 1], f32, tag="ones128")
            fp2all = big.tile([128, M_TILES], f32, tag="fp2all")
            cn2all = big.tile([128, M_TILES], f32, tag="cn2all")
            fpacc = big.tile([128, M_TILES * GN_TILES], f32, tag="fpacc")
            cnacc = big.tile([128, M_TILES * GN_TILES], f32, tag="cnacc")
            scrP = big.tile([128, GNT], f32, tag="scrP")
            scrN = big.tile([128, GNT], f32, tag="scrN")
            accT = big.tile([128, 2], f32, tag="accT")
            beps = big.tile([128, 1], f32, tag="beps")
            bpos = big.tile([128, 1], f32, tag="bpos")
            bneg = big.tile([128, 1], f32, tag="bneg")

            # src-side + const DMAs first (small; gate the critical chain)
            nc.sync.dma_start(nlW[:], cst_d.bitcast(f32r)[:])
            nc.sync.dma_start(lhsP[:], srcP_d.bitcast(f32r)[:])
            nc.sync.dma_start(lhsF[0:32, :], srcF_d[:])
            rtt_sb = big.tile([3, 4], f32r, tag="rtt")
            nc.sync.dma_start(rtt_sb[:], cst_d.bitcast(f32r)[40:43, :])
            praw = lhsP[0:3, :]
            ones_bf = big.tile([1, ROWS_PER_CORE], bf16, tag="ones_bf")
            nc.gpsimd.memset(ones_bf[:], 1.0)
            nc.sync.dma_start(lhsF[33:34, :], ones_bf[0:1, :])
            nc.gpsimd.memset(rhsF[32:33, :], 1.0)
            # preload the sqrt table set (Square lives in every set, so
            # this avoids a second 1.3us ACT_TABLE_LOAD in the tail)
            sqwarm = small.tile([1, 1], f32, tag="sqwarm")
            nc.scalar.activation(sqwarm[:], ones_bf[0:1, 0:1], AF.Sqrt)
            nc.gpsimd.memset(ones128[:], 1.0)
            nc.gpsimd.memset(beps[:], EPS)
            nc.gpsimd.memset(bpos[:], -POS_THRESH)
            nc.gpsimd.memset(bneg[:], NEG_THRESH)

            def tgt_dma(ch):
                sl = slice(ch * NT, (ch + 1) * NT)
                nc.sync.dma_start(rhsF[0:32, sl], tgtF_d[:, sl])
                nc.sync.dma_start(rhsP[:, sl], tgtP_d.bitcast(f32r)[:, sl])

            def tgt_dma2(cp):
                tgt_dma(2 * cp)
                tgt_dma(2 * cp + 1)

            # ---- src-side prep ----
            # pts chain (critical: gates the first psp1): rotate, square,
            # norm-matmul, land lhsP rows 0:4
            for ch in range(ROWS_PER_CORE // NT):
                sl = slice(ch * NT, (ch + 1) * NT)
                psrt = pf_p.tile([128, GNT], f32, tag="psf")
                psr = psrt[0:3, 0:NT]
                nc.tensor.matmul(out=psr, lhsT=rtt_sb[0:3, 0:3],
                                 rhs=praw[:, sl], start=True, stop=True)
                # 2V * (R p + t)
                nc.vector.tensor_scalar(
                    out=prot[:, sl], in0=psr,
                    scalar1=rtt_sb.bitcast(f32)[0:3, 3:4], scalar2=2.0 * V,
                    op0=A.add, op1=A.mult)
            nc.sync.dma_start(lhsP[0:3, :], prot.bitcast(f32r)[:])
            nc.vector.tensor_tensor(out=sqp[:], in0=prot[:], in1=prot[:],
                                    op=A.mult)
            for ch in range(ROWS_PER_CORE // NT):
                sl = slice(ch * NT, (ch + 1) * NT)
                psnt = pf_p.tile([128, GNT], f32, tag="psf")
                psn = psnt[0:1, 0:NT]
                nc.tensor.matmul(out=psn, lhsT=nlW[0:3, 1:2],
                                 rhs=sqp[:, sl], start=True, stop=True)
                stg = small.tile([1, NT], f32, tag="stg")
                nc.vector.tensor_copy(stg[:], psn)
                nc.sync.dma_start(lhsP[3:4, sl], stg.bitcast(f32r)[0:1, :])
            # feats norm row |a|^2 (bf16, from (-2a)^2 * 0.25)
            nc.vector.tensor_tensor(out=sqf[:], in0=lhsF[0:32, :],
                                    in1=lhsF[0:32, :], op=A.mult)
            for ch in range(ROWS_PER_CORE // NT):
                sl = slice(ch * NT, (ch + 1) * NT)
                psnt = pf_p.tile([128, GNT], f32, tag="psf")
                psn = psnt[0:1, 0:NT]
                nc.tensor.matmul(out=psn, lhsT=nlW[0:32, 0:1],
                                 rhs=sqf[0:32, sl], start=True, stop=True)
                stgb = small.tile([1, NT], bf16, tag="stgb")
                nc.vector.tensor_copy(stgb[:], psn)
                nc.sync.dma_start(lhsF[32:33, sl], stgb[0:1, :])

            # ---- tgt-side prep compute for one 512 chunk: squares of b
            # and q -> one K=35 norm matmul -> land rhsF row 32 (bf16) and
            # rhsP row 4 (f32r).  Split into two halves, spread through
            # the main loop so the Act queue never hiccups much ----
            def tgt_sqF(ch):
                sl = slice(ch * NT, (ch + 1) * NT)
                sq = sq_p.tile([35, NT], f32r, tag="sqt")
                nc.scalar.activation(sq[0:32, :], rhsF[0:32, sl], AF.Square)
                return sq

            def tgt_sqP(ch, sq):
                sl = slice(ch * NT, (ch + 1) * NT)
                nc.scalar.activation(sq[32:35, :], rhsP.bitcast(f32)[0:3, sl],
                                     AF.Square)
                return sq

            def tgt_sq(ch):
                return tgt_sqP(ch, tgt_sqF(ch))

            def tgt_norm(ch, sq):
                sl = slice(ch * NT, (ch + 1) * NT)
                psnt = pf_p.tile([128, GNT], f32, tag="psf")
                psnB = psnt[0:1, 0:NT]
                psnQ = psnt[0:1, NT:GNT]
                nc.tensor.matmul(out=psnB, lhsT=nlW[0:35, 2:3],
                                 rhs=sq[0:35, :], start=True, stop=True)
                nc.tensor.matmul(out=psnQ, lhsT=nlW[0:35, 3:4],
                                 rhs=sq[0:35, :], start=True, stop=True)
                stgb = small.tile([1, NT], bf16, tag="stgb2")
                nc.scalar.copy(stgb[:], psnB)
                nc.sync.dma_start(rhsF[33:34, sl], stgb[0:1, :])
                stg = small.tile([1, NT], f32, tag="stg2")
                nc.scalar.copy(stg[:], psnQ)
                nc.sync.dma_start(rhsP[4:5, sl], stg.bitcast(f32r)[0:1, :])

            for cp in range(2):
                tgt_dma2(cp)
            for ch in range(2):
                tgt_norm(ch, tgt_sq(ch))

            sq_pend = [None, None]

            def main_loop(_iv=None):
                for n in range(GN_TILES):
                    for m in range(M_TILES):
                        if repeat == 1 and n < 7:
                            if m == 0 and n < 6:
                                tgt_dma2(n + 2)
                            elif m == 1:
                                sq_pend[0] = tgt_sqF(2 * n + 2)
                            elif m == 2:
                                tgt_sqP(2 * n + 2, sq_pend[0])
                            elif m == 3:
                                tgt_norm(2 * n + 2, sq_pend[0])
                            elif m == 4:
                                sq_pend[1] = tgt_sqF(2 * n + 3)
                            elif m == 5:
                                tgt_sqP(2 * n + 3, sq_pend[1])
                            elif m == 6:
                                tgt_norm(2 * n + 3, sq_pend[1])
                        msl = slice(m * 128, (m + 1) * 128)
                        psf = pf_p.tile([128, GNT], f32, tag="psf")
                        psp1 = pp1_p.tile([128, GNT], f32, tag="psp1")
                        for g in range(2):
                            nsl = slice(n * GNT + g * NT,
                                        n * GNT + (g + 1) * NT)
                            gsl = slice(g * NT, (g + 1) * NT)
                            nc.tensor.matmul(out=psf[:, gsl],
                                             lhsT=lhsF[:, msl],
                                             rhs=rhsF[:, nsl],
                                             start=True, stop=True)
                        for g in range(2):
                            nsl = slice(n * GNT + g * NT,
                                        n * GNT + (g + 1) * NT)
                            gsl = slice(g * NT, (g + 1) * NT)
                            nc.tensor.matmul(out=psp1[:, gsl],
                                             lhsT=lhsP[0:6, msl],
                                             rhs=rhsP[0:6, nsl],
                                             start=True, stop=True)
                        fsb = fsb_p.tile([128, GNT], f32, tag="fsb")
                        nc.scalar.copy(fsb[:], psf[:])
                        c = m * GN_TILES + n
                        nc.vector._custom_dve(
                            pos_op, out=scrP[:], in0=psp1[:], in1=fsb[:],
                            s0=0.0, accum_out=fpacc[:, c:c + 1])
                        nc.vector._custom_dve(
                            neg_op, out=scrN[:], in0=psp1[:], in1=fsb[:],
                            s0=BIGF, s1=DSHIFT,
                            accum_out=cnacc[:, c:c + 1])
                        if n == GN_TILES - 1:
                            csl = slice(m * GN_TILES, (m + 1) * GN_TILES)
                            nc.vector.tensor_reduce(
                                out=fp2all[:, m:m + 1], in_=fpacc[:, csl],
                                op=A.max, axis=X)
                            nc.vector.tensor_reduce(
                                out=cn2all[:, m:m + 1], in_=cnacc[:, csl],
                                op=A.min, axis=X)

            if repeat == 1:
                main_loop()
            else:
                with tc.For_i(0, repeat, 1) as iv:
                    main_loop(iv)

            # ---- tail: sqrt / relu thresholds / partition sums ----
            fp = small.tile([128, M_TILES], f32, tag="fp")
            cn = small.tile([128, M_TILES], f32, tag="cn")
            nc.scalar.activation(fp[:], fp2all[:], AF.Sqrt, bias=beps[:])
            nc.scalar.activation(cn[:], cn2all[:], AF.Sqrt, bias=beps[:])
            pl = small.tile([128, M_TILES], f32, tag="pl")
            nl = small.tile([128, M_TILES], f32, tag="nl")
            nc.scalar.activation(pl[:], fp[:], AF.Relu, bias=bpos[:])
            nc.scalar.activation(nl[:], cn[:], AF.Relu, bias=bneg[:],
                                 scale=-1.0)
            nc.vector.tensor_reduce(out=accT[:, 0:1], in_=pl[:], op=A.add,
                                    axis=X)
            nc.vector.tensor_reduce(out=accT[:, 1:2], in_=nl[:], op=A.add,
                                    axis=X)
            psot = pf_p.tile([128, GNT], f32, tag="psf")
            pso = psot[0:2, 0:1]
            nc.tensor.matmul(out=pso, lhsT=accT[:], rhs=ones128[:],
                             start=True, stop=True)
            res_sb = small.tile([2, 1], f32, tag="res")
            nc.scalar.copy(res_sb[:], pso)
            nc.sync.dma_start(out_d[:], res_sb[:])

    nc.compile()
    return nc


def make_in_maps(src_pcd, tgt_pcd, src_feats, tgt_feats, correspondence,
                 rot, trans):
    """Host-side gather/shard/layout (indexing, transpose, exact constant
    scaling, dtype cast and constant fills only)."""
    import ml_dtypes
    bf16 = ml_dtypes.bfloat16
    ci = np.asarray(correspondence[:, 0]).astype(np.int64)
    cj = np.asarray(correspondence[:, 1]).astype(np.int64)
    src_pcd = np.asarray(src_pcd, np.float32)
    tgt_pcd = np.asarray(tgt_pcd, np.float32)
    src_feats = np.asarray(src_feats, np.float32)
    tgt_feats = np.asarray(tgt_feats, np.float32)

    # center pts at the box center: tf32 (fp32r) input rounding error is
    # relative to coordinate magnitude; |p-q|^2 is shift-invariant
    CEN = np.float32(0.1)

    tgtF = tgt_feats[cj].T.astype(bf16)
    # rows 32 (ones) and 33 (|b|^2) are device-side
    tgtP = np.zeros((KP, N_SEL), np.float32)
    tgtP[0:3] = tgt_pcd[cj].T - CEN
    tgtP[3] = 1.0
    # [4] = |q|^2 (device)
    tgtP[5] = np.float32(V) * np.float32(C1)

    srcF = (np.float32(-2.0) * src_feats[ci].T).astype(bf16)
    # rows 32 (|a|^2) and 33 (ones) are device-side
    srcP = np.zeros((KP, ROWS_PER_CORE * N_CORES), np.float32)
    srcP[0:3] = src_pcd[ci].T  # device applies rot/trans and 2V
    # [3] = -V|p^|^2 (device)
    srcP[4] = -np.float32(V)
    srcP[5] = 1.0



    # norm-term matmul weights:
    #   col0: |a|^2 from (-2a)^2 (K=32 over sqf)
    #   col1: -V|p^|^2 from (2Vp^)^2 (K=3 over sqp)
    #   col2: |b|^2 (rows 0:32 of sq chunk); col3: |q|^2 (rows 32:35)
    cst = np.zeros((43, 4), np.float32)
    cst[0:32, 0] = 0.25
    cst[0:3, 1] = np.float32(-1.0 / (4.0 * V))
    cst[0:32, 2] = 1.0
    cst[32:35, 3] = 1.0
    cst[40:43, 0:3] = np.asarray(rot, np.float32).T
    cst[40:43, 3] = np.asarray(trans, np.float32)[:, 0] - CEN

    in_maps = []
    for c in range(N_CORES):
        sl = slice(c * ROWS_PER_CORE, (c + 1) * ROWS_PER_CORE)
        in_maps.append({
            "srcF": np.ascontiguousarray(srcF[:, sl]),
            "srcP": np.ascontiguousarray(srcP[:, sl]),
            "tgtF": tgtF,
            "tgtP": tgtP,
            "cst": cst,
        })
    return in_maps


def combine_outputs(results):
    """Host-side unshard: sum per-core partial sums, divide by N."""
    tot = np.zeros(2, np.float32)
    for r in results:
        tot += r["out"][:, 0].astype(np.float32)
    loss = np.float32(tot[0] / np.float32(N_SEL) + tot[1] / np.float32(N_SEL))
    return np.float32(loss)


def kernel(src_pcd, tgt_pcd, src_feats, tgt_feats, correspondence, rot,
           trans):
    from concourse import bass_utils

    key = ("prog", 1)
    if key not in _PROGRAM_CACHE:
        _PROGRAM_CACHE[key] = build_program(repeat=1)
    nc = _PROGRAM_CACHE[key]
    in_maps = make_in_maps(src_pcd, tgt_pcd, src_feats, tgt_feats,
                           correspondence, rot, trans)
    res = bass_utils.run_bass_kernel_spmd(nc, in_maps,
                                          core_ids=list(range(N_CORES)))
    return combine_outputs(res.results)
